# revision 2
# baseline (speedup 1.0000x reference)
"""v3: same math as v2, restructured schedule.

- Consolidated DMAs (one per weight tensor, one per x token-chunk).
- Proj chunk c+1 and oproj(qb-1) are split into ~1.7us PE quanta and
  pumped as filler between QK score groups, so the PE stays busy while
  ACT's exp stream paces the psc PSUM ring.
- Transposes deferred two heads; oproj deferred one q-block.
"""

import sys
import numpy as np
from collections import deque

if "/opt/trn_rl_repo" not in sys.path:
    sys.path.insert(0, "/opt/trn_rl_repo")

import concourse.bass as bass
import concourse.tile as tile
from concourse import bacc
from concourse import mybir

F32 = mybir.dt.float32
F16 = mybir.dt.float16
F8 = mybir.dt.float8e4
DR = mybir.MatmulPerfMode.DoubleRow
EXP = mybir.ActivationFunctionType.Exp
SX, SW = 8.0, 256.0
DESCALE = 1.0 / (SX * SW)

P = 128
S, D = 2048, 1024
H8 = 8
HD = 64
NDT = D // P
NJT = 4
NTT = S // P
NQB = 8
QB = 256
CH = 512
NCH = S // CH
VROW = HD + 1
SCALE = 1.0 / 8.0


def plan_groups(qb):
    nfull = 2 * qb
    groups = []
    kt = 0
    while nfull - kt >= 4:
        g = [(i * QB, QB, kt + i, 0) for i in range(4)]
        groups.append((1024, g))
        kt += 4
    r = nfull - kt
    g = [(i * QB, QB, kt + i, 0) for i in range(r)]
    base = r * QB
    g.append((base, P, 2 * qb, 0))
    g.append((base + P, P, 2 * qb + 1, P))
    g.append((base + 2 * P, P, 2 * qb, P))
    groups.append((base + 384, g))
    return groups, base


def av_consume(qb, groups, base):
    fin = len(groups) - 1
    out = []
    for kt in range(2 * qb):
        gi, i = divmod(kt, 4)
        pos = i * QB
        out.append((kt, [(0, gi, pos), (1, gi, pos + P)]))
    out.append((2 * qb, [(0, fin, base), (1, fin, base + 2 * P)]))
    out.append((2 * qb + 1, [(1, fin, base + P)]))
    return out


def build_program(num_devices: int = 8, replicate: int = 1) -> bass.Bass:
    nc = bacc.Bacc(
        "TRN2", target_bir_lowering=False, debug=False, num_devices=num_devices
    )
    x8d = nc.dram_tensor("x8", [P, 8 * S], F8, kind="ExternalInput")
    xr8d = nc.dram_tensor("xr8", [P, 8 * S], F8, kind="ExternalInput")
    wq8d = nc.dram_tensor("wq8", [P, 4096], F8, kind="ExternalInput")
    wqr8d = nc.dram_tensor("wqr8", [P, 4096], F8, kind="ExternalInput")
    wk8d = nc.dram_tensor("wk8", [P, 4096], F8, kind="ExternalInput")
    wkr8d = nc.dram_tensor("wkr8", [P, 4096], F8, kind="ExternalInput")
    wv8d = nc.dram_tensor("wv8", [P, 4096], F8, kind="ExternalInput")
    wvr8d = nc.dram_tensor("wvr8", [P, 4096], F8, kind="ExternalInput")
    wo = nc.dram_tensor("wo", [512, D], F16, kind="ExternalInput")
    mask2 = nc.dram_tensor("mask2", [P, 2 * P], F16, kind="ExternalInput")
    ident = nc.dram_tensor("ident", [P, P], F16, kind="ExternalInput")
    out = nc.dram_tensor("out", [S, D], F16, kind="ExternalOutput")

    with tile.TileContext(nc) as tc:
        with (
            tc.tile_pool(name="res", bufs=1) as res,
            tc.tile_pool(name="exp", bufs=12) as exp_pool,
            tc.tile_pool(name="recp", bufs=4) as recp,
            tc.tile_pool(name="obp", bufs=2) as obp,
            tc.tile_pool(name="psca", bufs=2, space="PSUM") as pscap,
            tc.tile_pool(name="pscb", bufs=2, space="PSUM") as pscbp,
            tc.tile_pool(name="pacc", bufs=2, space="PSUM") as paccp,
        ):
          for _rep in range(replicate):
            # ---------------- resident SBUF tensors ----------------
            x8 = res.tile([P, 4 * 2 * S], F8, tag="x8", name="x8")
            xr8 = res.tile([P, 4 * 2 * S], F8, tag="xr8", name="xr8")
            wq8 = res.tile([P, 4 * 2 * 512], F8, tag="wq8", name="wq8")
            wqr8 = res.tile([P, 4 * 2 * 512], F8, tag="wqr8", name="wqr8")
            wk8 = res.tile([P, 4 * 2 * 512], F8, tag="wk8", name="wk8")
            wkr8 = res.tile([P, 4 * 2 * 512], F8, tag="wkr8", name="wkr8")
            wv8 = res.tile([P, 4 * 2 * 512], F8, tag="wv8", name="wv8")
            wvr8 = res.tile([P, 4 * 2 * 512], F8, tag="wvr8", name="wvr8")
            wo_sb = res.tile([P, NJT * D], F16, tag="wo", name="wo_sb")

            def v8(t):
                return t.rearrange("p (dp i s) -> p dp i s", dp=4, i=2)
            KT = res.tile([P, NJT * S], F16, tag="KT", name="KT")
            QT = res.tile([P, NJT * S], F16, tag="QT", name="QT")
            Vh = res.tile([P, NTT * H8 * VROW], F16, tag="Vh", name="Vh")
            ctx = res.tile([P, NTT * 512], F16, tag="ctx", name="ctx")
            ctxT = res.tile([P, NJT * S], F16, tag="ctxT", name="ctxT")
            mk = res.tile([P, 2 * P], F16, tag="mk", name="mk")
            idn = res.tile([P, P], F16, tag="idn", name="idn")

            # Consolidated DMAs, in first-use order.
            def dma_w8(dst, srcd):
                nc.sync.dma_start(out=dst, in_=srcd[:, :])

            def dma_x8(dst, srcd, c):
                nc.sync.dma_start(
                    out=dst.rearrange("p (a s) -> p a s", s=S)[
                        :, :, c * CH : (c + 1) * CH
                    ],
                    in_=srcd.rearrange("p (a s) -> p a s", s=S)[
                        :, :, c * CH : (c + 1) * CH
                    ],
                )

            def dma_weight(dst, srcd, dtiles):
                nc.sync.dma_start(
                    out=dst.rearrange("p (d w) -> p d w", d=dtiles),
                    in_=srcd.rearrange("(d p) w -> p d w", p=P),
                )

            dma_x8(x8, x8d, 0)
            dma_w8(wv8, wv8d)
            dma_w8(wvr8, wvr8d)
            dma_x8(xr8, xr8d, 0)
            dma_w8(wk8, wk8d)
            dma_w8(wkr8, wkr8d)
            dma_w8(wq8, wq8d)
            dma_w8(wqr8, wqr8d)
            dma_x8(x8, x8d, 1)
            dma_x8(xr8, xr8d, 1)
            dma_weight(wo_sb, wo, NJT)
            nc.sync.dma_start(out=mk, in_=mask2[:, :])
            nc.sync.dma_start(out=idn, in_=ident[:, :])
            dma_x8(x8, x8d, 2)
            dma_x8(xr8, xr8d, 2)
            dma_x8(x8, x8d, 3)
            dma_x8(xr8, xr8d, 3)

            ones_view = Vh.rearrange("p (t h v) -> p t h v", h=H8, v=VROW)[
                :, :, :, HD : HD + 1
            ]
            nc.vector.memset(ones_view, 1.0)

            # ---------------- quantum emitters ----------------
            def proj_kq(wpair, dstT, j, tlo, thi):
                # 3-pass fp8 error-feedback: x8@w8 + x8@wr8 + xr8@w8,
                # accumulated in one PSUM group at scale SX*SW.
                w8t, wr8t = wpair
                ps = pscbp.tile([P, thi - tlo], F32, tag="pscb", name="pk")
                idx = 0
                for xa, wa in ((x8, w8t), (x8, wr8t), (xr8, w8t)):
                    for dp in range(4):
                        nc.tensor.matmul(
                            ps,
                            v8(wa)[:, dp, :, j * P : (j + 1) * P],
                            v8(xa)[:, dp, :, tlo:thi],
                            start=(idx == 0),
                            stop=(idx == 11),
                            perf_mode=DR,
                        )
                        idx += 1
                nc.vector.tensor_scalar_mul(
                    dstT[:, j * S + tlo : j * S + thi], ps, DESCALE
                )

            def proj_v(t, half):
                ps = pscbp.tile([P, 256], F32, tag="pscb", name="pv")
                idx = 0
                for xa, wa in ((x8, wv8), (x8, wvr8), (xr8, wv8)):
                    for dp in range(4):
                        nc.tensor.matmul(
                            ps,
                            v8(xa)[:, dp, :, t * P : (t + 1) * P],
                            v8(wa)[:, dp, :, half * 256 : (half + 1) * 256],
                            start=(idx == 0),
                            stop=(idx == 11),
                            perf_mode=DR,
                        )
                        idx += 1
                dst = Vh.rearrange("p (t h v) -> p t h v", h=H8, v=VROW)[
                    :, t, 4 * half : 4 * (half + 1), 0:HD
                ]
                src = ps.rearrange("p (h v) -> p h v", v=HD)
                nc.vector.tensor_scalar_mul(dst, src, DESCALE)

            def emit_oproj(qt):
                ob = obp.tile([P, 1024], F16, tag="ob", name="ob")
                for half in range(2):
                    ps = pscbp.tile([P, 512], F32, tag="pscb", name="po")
                    for j in range(NJT):
                        nc.tensor.matmul(
                            ps,
                            ctxT[:, j * S + qt * P : j * S + (qt + 1) * P],
                            wo_sb[:, j * D + half * 512 : j * D + (half + 1) * 512],
                            start=(j == 0),
                            stop=(j == NJT - 1),
                        )
                    nc.vector.tensor_copy(ob[:, half * 512 : (half + 1) * 512], ps)
                nc.sync.dma_start(out=out[qt * P : (qt + 1) * P, :], in_=ob)

            fq = deque()
            pace = {"due": 0.0, "per": 1.0}

            def pump(n=1):
                for _ in range(n):
                    if not fq:
                        return
                    fq.popleft()()

            def pump_paced():
                # emit fillers at a rate that spreads the queue across the
                # stretch's remaining pump sites
                pace["due"] += pace["per"]
                while pace["due"] >= 1.0 and fq:
                    pace["due"] -= 1.0
                    fq.popleft()()

            # ---------------- attention emitters ----------------
            state = {}

            def emit_qk_group(h, qb, gi):
                j, off = h // 2, HD * (h % 2)
                qcol = qb * QB
                groups, base, exs = state[(h, qb)]
                cols, specs = groups[gi]
                sc = pscap.tile([P, cols], F32, tag="psca", name="sc")
                for pos, w, kt, qoff in specs:
                    nc.tensor.matmul(
                        sc[:, pos : pos + w],
                        KT[off : off + HD, j * S + kt * P : j * S + (kt + 1) * P],
                        QT[off : off + HD, j * S + qcol + qoff : j * S + qcol + qoff + w],
                        start=True,
                        stop=True,
                    )
                ex = exp_pool.tile([P, cols], F16, tag="ex", name="ex")
                nc.scalar.activation(ex, sc, EXP, scale=SCALE)
                if gi == len(groups) - 1:
                    nc.vector.tensor_mul(
                        ex[:, base : base + 2 * P], ex[:, base : base + 2 * P], mk
                    )
                exs.append(ex)

            def emit_av(h, qb):
                groups, base, exs = state.pop((h, qb))
                consume = av_consume(qb, groups, base)
                acc = [
                    paccp.tile([P, VROW], F32, tag="acc", name="acc0"),
                    paccp.tile([P, VROW], F32, tag="acc", name="acc1"),
                ]
                last_kt = (2 * qb, 2 * qb + 1)
                for kt, uses in consume:
                    vsl = Vh[:, kt * H8 * VROW + h * VROW : kt * H8 * VROW + (h + 1) * VROW]
                    for ai, gi, col in uses:
                        nc.tensor.matmul(
                            acc[ai],
                            exs[gi][:, col : col + P],
                            vsl,
                            start=(kt == 0),
                            stop=(kt == last_kt[ai]),
                        )
                for ai in range(2):
                    qt = 2 * qb + ai
                    rec = recp.tile([P, 1], F32, tag="rec", name="rec")
                    nc.vector.reciprocal(rec, acc[ai][:, HD : HD + 1])
                    nc.vector.tensor_scalar_mul(
                        ctx[:, qt * 512 + h * HD : qt * 512 + (h + 1) * HD],
                        acc[ai][:, 0:HD],
                        rec,
                    )

            def emit_transpose(hp, qb):
                for qt in (2 * qb, 2 * qb + 1):
                    tr = pscbp.tile([P, P], F16, tag="pscb", name="tr")
                    nc.tensor.transpose(
                        tr, ctx[:, qt * 512 + hp * P : qt * 512 + (hp + 1) * P], idn
                    )
                    nc.vector.tensor_copy(
                        ctxT[:, hp * S + qt * P : hp * S + (qt + 1) * P], tr
                    )

            # ---------------- main schedule ----------------
            # chunk 0 solid, ordered to match DMA arrival
            WKP, WQP = (wk8, wkr8), (wq8, wqr8)
            proj_v(0, 0)
            proj_v(0, 1)
            for j in range(NJT):
                for w in range(2):
                    proj_kq(WKP, KT, j, w * 256, (w + 1) * 256)
            for j in range(NJT):
                for w in range(2):
                    proj_kq(WQP, QT, j, w * 256, (w + 1) * 256)
            for t in range(1, 4):
                proj_v(t, 0)
                proj_v(t, 1)

            for qb in range(NQB):
                # force-drain: queued proj quanta must land before a
                # stretch whose QK/AV reads them (emission order is
                # per-engine execution order)
                if qb % 2 == 0 or qb == NQB - 1:
                    while fq:
                        pump(1)
                # proj fillers for the NEXT chunk, token-sliced so work
                # needed latest lands latest (enriching the ACT-bound
                # qb6/qb7 stretches with PE work)
                def qk_quanta(wpair, dstT, tlo, thi):
                    for j in range(NJT):
                        for a in range(tlo, thi, 256):
                            fq.append(lambda j=j, a=a: proj_kq(wpair, dstT, j, a, a + 256))

                def v_quanta(*tiles):
                    for t in tiles:
                        fq.append(lambda t=t: proj_v(t, 0))
                        fq.append(lambda t=t: proj_v(t, 1))
                if qb == 0:
                    qk_quanta((wk8, wkr8), KT, CH, 2 * CH)
                    v_quanta(4, 5)
                elif qb == 1:
                    qk_quanta((wq8, wqr8), QT, CH, 2 * CH)
                    v_quanta(6, 7)
                elif qb == 2:
                    qk_quanta((wk8, wkr8), KT, 2 * CH, 3 * CH)
                    v_quanta(8, 9)
                elif qb == 3:
                    qk_quanta((wq8, wqr8), QT, 2 * CH, 3 * CH)
                    v_quanta(10, 11)
                elif qb == 4:
                    qk_quanta((wk8, wkr8), KT, 3 * CH, 3 * CH + 256)
                    v_quanta(12, 13)
                elif qb == 5:
                    qk_quanta((wq8, wqr8), QT, 3 * CH, 3 * CH + 256)
                elif qb == 6:
                    qk_quanta((wk8, wkr8), KT, 3 * CH + 256, 4 * CH)
                    qk_quanta((wq8, wqr8), QT, 3 * CH + 256, 4 * CH)
                    v_quanta(14, 15)
                # transposes of the previous q-block become fillers here
                if qb > 0:
                    fq.append(lambda q=qb - 1: (emit_transpose(0, q), emit_transpose(1, q)))
                    fq.append(lambda q=qb - 1: (emit_transpose(2, q), emit_transpose(3, q)))
                # oproj spread over the late stretches
                OPROJ_AT = {5: (0, 1, 2, 3), 6: (4, 5, 6, 7),
                            7: (8, 9, 10, 11, 12, 13)}
                for qt in OPROJ_AT.get(qb, ()):
                    fq.append(lambda qt=qt: emit_oproj(qt))

                lag = 2
                ngroups = len(plan_groups(qb)[0])
                sites = H8 * max(1, ngroups - 1) + lag
                pace["per"] = len(fq) / sites if sites else 1.0
                pace["due"] = 0.0
                for h in range(H8):
                    groups, base = plan_groups(qb)
                    state[(h, qb)] = (groups, base, [])
                    emit_qk_group(h, qb, 0)
                    if h >= lag:
                        emit_av(h - lag, qb)
                    for gi in range(1, len(groups)):
                        emit_qk_group(h, qb, gi)
                        pump_paced()
                    if ngroups == 1:
                        pump_paced()
                if qb == NQB - 1:
                    # tail: shorten the final dependency chain
                    emit_transpose(0, qb)
                    emit_transpose(1, qb)
                    emit_transpose(2, qb)
                for h in range(H8 - lag, H8):
                    emit_av(h, qb)
                    pump_paced()

            # drain
            while fq:
                pump(1)
            emit_transpose(3, NQB - 1)
            emit_oproj(14)
            emit_oproj(15)

    return nc


# ---------------------------------------------------------------------------
# Host-side sharding / assembly
# ---------------------------------------------------------------------------

NCORES = 8
B = 4


def _pair_pack(a, ncols):
    """[1024, ncols] -> [128, 8*ncols] partition-major with d-chunk pairs
    interleaved for DoubleRow: col (dp*2 + i)*ncols + w = a[(2*dp+i)*128+p, w]."""
    return np.ascontiguousarray(
        a.reshape(4, 2, 128, ncols).transpose(2, 0, 1, 3).reshape(128, 8 * ncols)
    )


def _q8(a, scale, f8dt):
    return np.asarray(a * scale, dtype=np.float32).astype(f8dt)


def _shard_inputs(x, Wq, Wk, Wv, Wo, bo):
    import ml_dtypes
    F8NP = ml_dtypes.float8_e4m3fn
    SX, SW = 8.0, 256.0
    x = np.asarray(x, np.float32)
    Wo = np.asarray(Wo, np.float32)
    k = np.arange(P)[:, None]
    q = np.arange(P)[None, :]
    tri = (k <= q).astype(np.float16)
    mask2 = np.concatenate([tri, tri], axis=1)
    ident = np.eye(P, dtype=np.float16)

    def wsplit(W, g):
        Wg = np.asarray(W, np.float32)[:, g * 512 : (g + 1) * 512]
        w8 = _q8(Wg, SW, F8NP)
        wr = Wg - w8.astype(np.float32) / SW
        wr8 = _q8(wr, SW, F8NP)
        return _pair_pack(w8, 512), _pair_pack(wr8, 512)

    ws = {}
    for g in range(2):
        ws[g] = {}
        for nm, W in (("q", Wq), ("k", Wk), ("v", Wv)):
            w8p, wr8p = wsplit(W, g)
            ws[g][f"w{nm}8"] = w8p
            ws[g][f"w{nm}r8"] = wr8p
        ws[g]["wo"] = np.ascontiguousarray(
            Wo[g * 512 : (g + 1) * 512, :]
        ).astype(np.float16)

    in_maps = []
    for c in range(NCORES):
        b, g = c // 2, c % 2
        xT = x[b].T
        x8 = _q8(xT, SX, F8NP)
        xr = xT - x8.astype(np.float32) / SX
        xr8 = _q8(xr, SX, F8NP)
        in_maps.append({
            "x8": _pair_pack(x8, S),
            "xr8": _pair_pack(xr8, S),
            **ws[g],
            "mask2": mask2,
            "ident": ident,
        })
    return in_maps


_NC_CACHE = {}


def _get_program():
    if "nc" not in _NC_CACHE:
        nc = build_program(num_devices=NCORES)
        nc.compile()
        _NC_CACHE["nc"] = nc
    return _NC_CACHE["nc"]


def kernel(x, Wq, Wk, Wv, Wo, bo):
    """Full-input, full-output causal MHA on 8 NeuronCores."""
    from concourse.bass_utils import run_bass_kernel_spmd

    nc = _get_program()
    in_maps = _shard_inputs(x, Wq, Wk, Wv, Wo, bo)
    res = run_bass_kernel_spmd(nc, in_maps, list(range(NCORES)))
    bo32 = np.asarray(bo, np.float32)
    out = np.zeros((B, S, D), np.float32)
    for b in range(B):
        out[b] = (
            res.results[2 * b]["out"].astype(np.float32)
            + res.results[2 * b + 1]["out"].astype(np.float32)
            + bo32
        )
    return out
